# revision 12
# baseline (speedup 1.0000x reference)
"""CoAttention (BiDAF-style) + depthwise-separable conv, Trainium2 Bass kernel.

Shapes (hardcoded): B=32, D=128, C_LEN=1024, Q_LEN=256.
Sharding: pure data-parallel over batch, 4 batches per core on 8 cores.

Math (masks enter only as additive -1e30 terms; row/col biases that are
constant along the softmax axis cancel, so S is never materialized with
both biases):
  S0[i,j]   = sum_k C_t[i,k] w3[k] Q_t[j,k]
  cb[j]     = Q_t[j].w2 (+ mask bias), rb[i] = C_t[i].w1 (+ mask bias)
  S_bar     = softmax_j(S0 + cb[j])      (rb cancels)
  S_bbar    = softmax_i(S0 + rb[i])      (cb cancels)
  A   = S_bar @ Q_t          (computed transposed: A^T, k-part x i-free)
  T   = S_bbar^T @ C_t       (j-part x k-free)
  Bm  = S_bar @ T            (computed transposed: Bm^T)
  x   = [C_t; A; C_t*A; C_t*Bm] channels (4*128, i)  -> depthwise conv5 -> pw conv
Softmax max-subtraction is skipped (|S| is O(5), exp is safe in fp32).
"""

import math
import os
from contextlib import ExitStack

import numpy as np

# The axon NTFF profile hook is not available in this container; a
# BASS_TRACE=1 leaking into the environment would crash the run path.
os.environ["BASS_NEVER_TRACE"] = "1"

import concourse.bass as bass
import concourse.mybir as mybir
import concourse.tile as tile
from concourse import bacc
from concourse.bass_utils import run_bass_kernel_spmd
from concourse.masks import make_identity

B, D, CL, QL = 32, 128, 1024, 256
NCORES = 8
BPC = B // NCORES  # batches per core
F32 = mybir.dt.float32
F32R = mybir.dt.float32r
BF16 = mybir.dt.bfloat16
AF = mybir.ActivationFunctionType
OP = mybir.AluOpType

NT_I = CL // 128  # 8 i-tiles
NT_J = QL // 128  # 2 j-tiles
NCH = CL // 512   # 2 n-chunks of 512

# knob: dtype used for matmul operands. float32r streams at 1 cycle/row
# (vs 4 for float32) but requires producer-side rounding; float32 is exact.
MM_DT = F32


def _mm(ap):
    """View an fp32 AP with the matmul operand dtype."""
    if MM_DT is F32R:
        return ap.bitcast(F32R)
    return ap


def build_kernel(wc_np: np.ndarray, pwT_np: np.ndarray):
    nc = bacc.Bacc("TRN2", target_bir_lowering=False, debug=False, num_devices=NCORES)

    C_in = nc.dram_tensor("C", [BPC, D, CL], F32, kind="ExternalInput")
    Q_in = nc.dram_tensor("Q", [BPC, D, QL], F32, kind="ExternalInput")
    cmb_in = nc.dram_tensor("cmb", [BPC, D, NT_I], F32, kind="ExternalInput")
    qmb_in = nc.dram_tensor("qmb", [BPC, D, NT_J], F32, kind="ExternalInput")
    out_d = nc.dram_tensor("out", [BPC, D, CL], F32, kind="ExternalOutput")

    wc_d = nc.inline_tensor(wc_np, "wc")      # (128, 25) packed consts
    pwT_d = nc.inline_tensor(pwT_np, "pwT")   # (512, 128) pw weights^T

    with tile.TileContext(nc) as tc, ExitStack() as ctx:
        consts = ctx.enter_context(tc.tile_pool(name="consts", bufs=1))
        sb = ctx.enter_context(tc.tile_pool(name="sb", bufs=2))
        psb = ctx.enter_context(tc.tile_pool(name="psb", bufs=3, space="PSUM"))
        pss = ctx.enter_context(tc.tile_pool(name="pss", bufs=2, space="PSUM"))

        wc = consts.tile([D, 25], F32)
        nc.sync.dma_start(out=wc, in_=wc_d[:, :])
        w1 = wc[:, 0:1]
        w2 = wc[:, 1:2]
        w3 = wc[:, 2:3]
        ones_col = wc[:, 3:4]
        fbias = wc[:, 4:5]

        pwT = consts.tile([D, 4, D], F32)
        nc.sync.dma_start(out=pwT, in_=pwT_d.rearrange("(g p) d -> p g d", p=D))
        ident = consts.tile([D, D], F32)
        make_identity(nc, ident)

        for b in range(BPC):
            # ---- loads (C goes into a border-padded tile: conv group 0) ----
            cbp = sb.tile([D, CL + 4], F32, tag="cbp")
            nc.gpsimd.memset(cbp[:, 0:2], 0.0)
            nc.gpsimd.memset(cbp[:, CL + 2 : CL + 4], 0.0)
            nc.sync.dma_start(out=cbp[:, 2 : CL + 2], in_=C_in[b])
            cb = cbp[:, 2 : CL + 2]
            qb = sb.tile([D, QL], F32, tag="qb")
            nc.sync.dma_start(out=qb, in_=Q_in[b])
            cmbt = sb.tile([D, NT_I], F32, tag="cmbt")
            nc.sync.dma_start(out=cmbt, in_=cmb_in[b])
            qmbt = sb.tile([D, NT_J], F32, tag="qmbt")
            nc.sync.dma_start(out=qmbt, in_=qmb_in[b])

            # ---- Qw3 = Q * w3 (per-partition scalar) ----
            qw3 = sb.tile([D, QL], F32, tag="qw3")
            nc.vector.tensor_scalar_mul(qw3, qb, w3)

            # ---- cb_col = Q_t @ w2 per j-tile, + mask bias ----
            ps_cb = pss.tile([D, NT_J], F32, tag="small")
            for jt in range(NT_J):
                nc.tensor.matmul(
                    ps_cb[:, jt : jt + 1],
                    _mm(qb[:, jt * 128 : (jt + 1) * 128]),
                    _mm(w2),
                    start=True,
                    stop=True,
                )
            cbm = sb.tile([D, NT_J], F32, tag="cbm")
            nc.vector.tensor_add(cbm, ps_cb, qmbt)

            # ---- S0^T (j-part, i-free) and E^T = exp(S0^T + cb[j]) ----
            et = []
            for jt in range(NT_J):
                s0t = psb.tile([D, CL], F32, tag="big")
                for n in range(NCH):
                    nc.tensor.matmul(
                        s0t[:, n * 512 : (n + 1) * 512],
                        _mm(qw3[:, jt * 128 : (jt + 1) * 128]),
                        _mm(cb[:, n * 512 : (n + 1) * 512]),
                        start=True,
                        stop=True,
                    )
                e = sb.tile([D, CL], F32, tag="et")
                nc.scalar.activation(e, s0t, AF.Exp, bias=cbm[:, jt : jt + 1])
                et.append(e)

            # ---- S0 i-part (for S_bbar) + row bias columns ----
            ps_rb = pss.tile([D, NT_I], F32, tag="small")
            s0ip = []
            for h in range(2):
                s0ip.append(psb.tile([D, CL], F32, tag="big", name=f"s0ip{h}"))
            for m in range(NT_I):
                h, m4 = divmod(m, 4)
                nc.tensor.matmul(
                    s0ip[h][:, m4 * 256 : (m4 + 1) * 256],
                    _mm(cb[:, m * 128 : (m + 1) * 128]),
                    _mm(qw3),
                    start=True,
                    stop=True,
                )
                nc.tensor.matmul(
                    ps_rb[:, m : m + 1],
                    _mm(cb[:, m * 128 : (m + 1) * 128]),
                    _mm(w1),
                    start=True,
                    stop=True,
                )
            rbm = sb.tile([D, NT_I], F32, tag="rbm")
            nc.vector.tensor_add(rbm, ps_rb, cmbt)
            exprb = sb.tile([D, NT_I], F32, tag="exprb")
            nc.scalar.activation(exprb, rbm, AF.Exp)
            sbb = []
            for h in range(2):
                s = sb.tile([D, CL], F32, tag="sbb")
                nc.scalar.activation(s, s0ip[h], AF.Exp)
                sbb.append(s)

            # ---- rs = sum_j E^T  (ones-matmul), r = 1/rs, broadcast ----
            rs = [
                pss.tile([1, 512], F32, tag="small", name=f"rs{n}") for n in range(NCH)
            ]
            for n in range(NCH):
                for jt in range(NT_J):
                    nc.tensor.matmul(
                        rs[n][0:1, :],
                        _mm(ones_col),
                        _mm(et[jt][:, n * 512 : (n + 1) * 512]),
                        start=(jt == 0),
                        stop=(jt == NT_J - 1),
                    )
            rrow = sb.tile([1, CL], F32, tag="rrow")
            for n in range(NCH):
                nc.vector.reciprocal(rrow[0:1, n * 512 : (n + 1) * 512], rs[n][0:1, :])
            Rb = sb.tile([D, CL], F32, tag="Rb")
            nc.gpsimd.partition_broadcast(Rb, rrow)

            # ---- C^T tiles via PE transpose, scaled by exp(rb) ----
            # cbt[:, ich, 0:128] = exp(rb_i) * C_t[i, :]; col 128 = exp(rb_i)
            cbt = sb.tile([D, NT_I, 129], F32, tag="cbt")
            for ich in range(NT_I):
                pt = pss.tile([D, D], F32, tag="small")
                nc.tensor.transpose(pt, cb[:, ich * 128 : (ich + 1) * 128], ident)
                nc.scalar.mul(cbt[:, ich, 0:128], pt, exprb[:, ich : ich + 1])
                nc.gpsimd.tensor_copy(cbt[:, ich, 128:129], exprb[:, ich : ich + 1])

            # ---- T = S_bbar^T @ C_t with built-in denominator column ----
            tT = []
            for jt in range(NT_J):
                ps_t = pss.tile([D, 129], F32, tag="small")
                for ich in range(NT_I):
                    h, m4 = divmod(ich, 4)
                    nc.tensor.matmul(
                        ps_t,
                        _mm(sbb[h][:, m4 * 256 + jt * 128 : m4 * 256 + (jt + 1) * 128]),
                        _mm(cbt[:, ich, :]),
                        start=(ich == 0),
                        stop=(ich == NT_I - 1),
                    )
                rt = sb.tile([D, 1], F32, tag="rt")
                nc.vector.reciprocal(rt, ps_t[:, 128:129])
                t_sb = sb.tile([D, D], F32, tag="tsb")
                nc.scalar.mul(t_sb, ps_t[:, 0:128], rt)
                tT.append(t_sb)

            # ---- Q_t tiles via PE transpose ----
            qbt = sb.tile([D, NT_J, D], F32, tag="qbt")
            for jt in range(NT_J):
                pt2 = pss.tile([D, D], F32, tag="small")
                nc.tensor.transpose(pt2, qb[:, jt * 128 : (jt + 1) * 128], ident)
                nc.scalar.copy(qbt[:, jt, :], pt2)

            # ---- A^T and Bm^T (contract over j) ----
            ps_a = psb.tile([D, CL], F32, tag="big")
            for n in range(NCH):
                for jt in range(NT_J):
                    nc.tensor.matmul(
                        ps_a[:, n * 512 : (n + 1) * 512],
                        _mm(qbt[:, jt, :]),
                        _mm(et[jt][:, n * 512 : (n + 1) * 512]),
                        start=(jt == 0),
                        stop=(jt == NT_J - 1),
                    )
            ps_b = psb.tile([D, CL], F32, tag="big")
            for n in range(NCH):
                for jt in range(NT_J):
                    nc.tensor.matmul(
                        ps_b[:, n * 512 : (n + 1) * 512],
                        _mm(tT[jt]),
                        _mm(et[jt][:, n * 512 : (n + 1) * 512]),
                        start=(jt == 0),
                        stop=(jt == NT_J - 1),
                    )

            # ---- conv input channel groups (padded for the 5-tap conv) ----
            g1 = sb.tile([D, CL + 4], F32, tag="g1")
            g2 = sb.tile([D, CL + 4], F32, tag="g2")
            g3 = sb.tile([D, CL + 4], F32, tag="g3")
            for g in (g1, g2, g3):
                nc.gpsimd.memset(g[:, 0:2], 0.0)
                nc.gpsimd.memset(g[:, CL + 2 : CL + 4], 0.0)
            tmp3 = sb.tile([D, CL], F32, tag="tmp3")
            nc.vector.tensor_mul(g1[:, 2 : CL + 2], ps_a, Rb)
            nc.vector.tensor_mul(g2[:, 2 : CL + 2], g1[:, 2 : CL + 2], cb)
            nc.vector.tensor_mul(tmp3, ps_b, Rb)
            nc.vector.tensor_mul(g3[:, 2 : CL + 2], tmp3, cb)

            # ---- depthwise conv5 + pointwise conv (fused bias at the end) ----
            ps_o = psb.tile([D, CL], F32, tag="big")
            for g, xg in enumerate((cbp, g1, g2, g3)):
                dwo = sb.tile([D, CL], F32, tag="dwo")
                wcol = lambda t: wc[:, 5 + g * 5 + t : 6 + g * 5 + t]
                nc.vector.tensor_scalar_mul(dwo, xg[:, 0:CL], wcol(0))
                for t in range(1, 5):
                    eng = nc.vector
                    eng.scalar_tensor_tensor(
                        dwo, xg[:, t : t + CL], wcol(t), dwo, OP.mult, OP.add
                    )
                for n in range(NCH):
                    nc.tensor.matmul(
                        ps_o[:, n * 512 : (n + 1) * 512],
                        _mm(pwT[:, g, :]),
                        _mm(dwo[:, n * 512 : (n + 1) * 512]),
                        start=(g == 0),
                        stop=(g == 3),
                    )

            outsb = sb.tile([D, CL], F32, tag="outsb")
            nc.scalar.activation(outsb, ps_o, AF.Identity, bias=fbias)
            nc.sync.dma_start(out=out_d[b], in_=outsb)

    nc.compile()
    return nc


def _host_prep(c_mask, q_mask, W0, dw_w, dw_b, pw_w, pw_b):
    w1, w2, w3 = W0[:D], W0[D : 2 * D], W0[2 * D :]
    wc = np.zeros((D, 25), np.float32)
    wc[:, 0] = w1
    wc[:, 1] = w2
    wc[:, 2] = w3
    wc[:, 3] = 1.0
    pw = pw_w[:, :, 0].astype(np.float32)  # (128, 512)
    wc[:, 4] = pw @ dw_b + pw_b
    dw = dw_w[:, 0, :].reshape(4, D, 5).astype(np.float32)
    wc[:, 5:25] = dw.transpose(1, 0, 2).reshape(D, 20)
    pwT = np.ascontiguousarray(pw.T)  # (512, 128)
    cmb = np.ascontiguousarray(
        ((c_mask - 1.0) * 1e30).reshape(B, NT_I, D).transpose(0, 2, 1)
    ).astype(np.float32)
    qmb = np.ascontiguousarray(
        ((q_mask - 1.0) * 1e30).reshape(B, NT_J, D).transpose(0, 2, 1)
    ).astype(np.float32)
    return wc, pwT, cmb, qmb


def kernel(C, Q, c_mask, q_mask, W0, dw_w, dw_b, pw_w, pw_b):
    C = np.ascontiguousarray(np.asarray(C, np.float32))
    Q = np.ascontiguousarray(np.asarray(Q, np.float32))
    wc, pwT, cmb, qmb = _host_prep(
        np.asarray(c_mask, np.float32),
        np.asarray(q_mask, np.float32),
        np.asarray(W0, np.float32),
        np.asarray(dw_w, np.float32),
        np.asarray(dw_b, np.float32),
        np.asarray(pw_w, np.float32),
        np.asarray(pw_b, np.float32),
    )
    nc = build_kernel(wc, pwT)
    in_maps = []
    for c in range(NCORES):
        sl = slice(c * BPC, (c + 1) * BPC)
        in_maps.append(
            {
                "C": np.ascontiguousarray(C[sl]),
                "Q": np.ascontiguousarray(Q[sl]),
                "cmb": np.ascontiguousarray(cmb[sl]),
                "qmb": np.ascontiguousarray(qmb[sl]),
            }
        )
    res = run_bass_kernel_spmd(nc, in_maps, core_ids=list(range(NCORES)))
    global LAST_RESULT, LAST_NC, LAST_IN_MAPS
    LAST_RESULT, LAST_NC, LAST_IN_MAPS = res, nc, in_maps
    out = np.concatenate([r["out"] for r in res.results], axis=0)
    return out.astype(np.float32)


LAST_RESULT = None
LAST_NC = None
LAST_IN_MAPS = None
